# revision 30
# baseline (speedup 1.0000x reference)
"""FP64->FP32 bit-circuit converter kernel for Trainium2 (8 NeuronCores).

Input:  fp64_pulse (1048576, 64) float32 of {0,1} bits (fp64, MSB first).
Output: (1048576, 32) float32 of {0,1} bits (fp32 conversion result).

Strategy (pure data parallel over batch, 131072 rows/core):
  - batch-2D layout: 128 partitions x 1024 inner rows per core,
    supertiles of NF inner rows ([128, NF*64] input tiles),
  - bit packing into integers via one broadcast-weight multiply and
    innermost-axis reduces (exp_val, mant_int, sticky_sum),
  - mantissa round-to-nearest-even done by fp32 hardware: adding
    frac = 0.5*R + 0.25*S to the 24-bit int (2^23 + mant_int) rounds
    exactly like the reference ripple-adder circuit,
  - special values (nan/inf/overflow/underflow) folded in as value-level
    muxes, then output bits extracted with int32 (shift, and) ops.
"""
import numpy as np

from concourse import bacc, mybir
from concourse.tile import TileContext
from concourse.bass_utils import run_bass_kernel_spmd

AOT = mybir.AluOpType
F32 = mybir.dt.float32
BF16 = mybir.dt.bfloat16
I32 = mybir.dt.int32

B = 1_048_576
N_CORES = 8
B_CORE = B // N_CORES          # 131072
P = 128                        # partitions
NI = B_CORE // P               # 1024 inner rows per partition
NF = 128                       # inner rows per supertile
N_ST = NI // NF                # supertiles per core
D_IN = 64
D_OUT = 32

# weight row: col 0 sign (unused), cols 1..11 exp MSB-first (2^10..2^0),
# cols 12..34 mant bits 0..22 MSB-first (2^22..2^0), rest 0.
_w_row = np.zeros(D_IN, np.float32)
_w_row[1:12] = [2.0 ** (10 - k) for k in range(11)]
_w_row[12:35] = [2.0 ** (22 - k) for k in range(23)]
import ml_dtypes
WCONST = np.broadcast_to(_w_row, (P, D_IN)).astype(ml_dtypes.bfloat16).copy()

_CACHE = {}


def _build():
    nc = bacc.Bacc("TRN2")
    x = nc.dram_tensor("x", [B_CORE, D_IN], F32, kind="ExternalInput")
    w = nc.dram_tensor("w", [P, D_IN], BF16, kind="ExternalInput")
    y = nc.dram_tensor("y", [B_CORE, D_OUT], F32, kind="ExternalOutput")

    x_r = x.ap().rearrange("(p n) d -> p (n d)", p=P)   # [128, NI*64]
    y_r = y.ap().rearrange("(p n) d -> p (n d)", p=P)   # [128, NI*32]

    with TileContext(nc) as tc:
        with (
            tc.tile_pool(name="consts", bufs=1) as consts,
            tc.tile_pool(name="io", bufs=2) as io,
            tc.tile_pool(name="mid", bufs=2) as mid,
            tc.tile_pool(name="sc", bufs=3) as sc,
        ):
            wt = consts.tile([P, D_IN], BF16)
            nc.sync.dma_start(wt[:, :], w.ap())

            # small first/last supertiles shorten the DMA head/tail ramp
            schedule = [32, 96] + [NF] * (N_ST - 2) + [96, 32]
            assert sum(schedule) == NI
            off = 0
            for nf in schedule:
                xin = io.tile([P, nf * D_IN], F32, tag="xin")
                nc.sync.dma_start(
                    xin[:, :], x_r[:, off * D_IN:(off + nf) * D_IN])

                xv = xin[:, :].rearrange("p (n d) -> p n d", d=D_IN)

                # --- weighted pack: mult by broadcast weights + reduces ---
                wb = wt[:, 1:35].unsqueeze(1).broadcast_to([P, nf, 34])
                xw = mid.tile([P, nf * 34], F32, tag="xw")
                xwv = xw[:, :].rearrange("p (n d) -> p n d", d=34)
                nc.vector.tensor_tensor(xwv, xv[:, :, 1:35], wb, AOT.mult)

                exp_val_t = sc.tile([P, nf], F32, tag="exp_val")
                nc.vector.tensor_reduce(
                    exp_val_t[:, :].unsqueeze(2), xwv[:, :, 0:11],
                    mybir.AxisListType.X, AOT.add)
                mant_int_t = sc.tile([P, nf], F32, tag="mant_int")
                nc.vector.tensor_reduce(
                    mant_int_t[:, :].unsqueeze(2), xwv[:, :, 11:34],
                    mybir.AxisListType.X, AOT.add)
                sticky_t = sc.tile([P, nf], F32, tag="sticky")
                nc.vector.tensor_reduce(
                    sticky_t[:, :].unsqueeze(2), xv[:, :, 36:64],
                    mybir.AxisListType.X, AOT.add)
                exp_val = exp_val_t[:, :]
                mant_int = mant_int_t[:, :]
                sticky = sticky_t[:, :]
                Rbit = xv[:, :, 35]
                sign = xv[:, :, 0]

                # --- mantissa RNE via hw fp32 add ---
                fr2 = sc.tile([P, nf], BF16, tag="fr2")
                nc.any.tensor_scalar(fr2[:, :], sticky, 1.0, 0.25,
                                     AOT.is_ge, AOT.mult)
                frac = sc.tile([P, nf], BF16, tag="frac")
                nc.vector.scalar_tensor_tensor(frac[:, :], Rbit, 0.5, fr2[:, :],
                                               AOT.mult, AOT.add)
                Mr = sc.tile([P, nf], F32, tag="Mr")
                nc.vector.scalar_tensor_tensor(Mr[:, :], mant_int,
                                               float(2 ** 23), frac[:, :],
                                               AOT.add, AOT.add)
                c_m = sc.tile([P, nf], BF16, tag="c_m")
                nc.any.tensor_scalar(c_m[:, :], Mr[:, :], float(2 ** 24),
                                     None, AOT.is_ge)
                # Mval = Mr - 2^23*c_m  (in [2^23, 2^24); bit 23 never read)
                Mval = sc.tile([P, nf], F32, tag="Mval")
                nc.vector.scalar_tensor_tensor(Mval[:, :], c_m[:, :],
                                               float(-(2 ** 23)), Mr[:, :],
                                               AOT.mult, AOT.add)

                # --- exponent value T2 = exp_val + c_m + 1152 ---
                T2 = sc.tile([P, nf], F32, tag="T2")
                nc.vector.scalar_tensor_tensor(T2[:, :], c_m[:, :], 1152.0,
                                               exp_val, AOT.add, AOT.add)

                # --- specials ---
                over = sc.tile([P, nf], BF16, tag="over")
                nc.any.tensor_scalar(over[:, :], exp_val, 1151.0, None,
                                     AOT.is_ge)
                under = sc.tile([P, nf], BF16, tag="under")
                nc.any.tensor_scalar(under[:, :], exp_val, 897.0, None,
                                     AOT.is_lt)
                # m_any = (mant_int + 2*frac) >= 0.5  (frac = 0.5R + 0.25S)
                ms1 = sc.tile([P, nf], F32, tag="ms1")
                nc.vector.scalar_tensor_tensor(ms1[:, :], frac[:, :], 2.0,
                                               mant_int, AOT.mult, AOT.add)
                m_any = sc.tile([P, nf], F32, tag="m_any")
                nc.any.tensor_scalar(m_any[:, :], ms1[:, :], 0.5, None, AOT.is_ge)
                is_max = sc.tile([P, nf], BF16, tag="is_max")
                nc.any.tensor_scalar(is_max[:, :], exp_val, 2047.0, None,
                                     AOT.is_ge)
                # nan = is_max AND m_any, via sum >= 2 (fused into nv below)
                nan2 = sc.tile([P, nf], F32, tag="nan2")
                nc.vector.tensor_tensor(nan2[:, :], is_max[:, :], m_any[:, :],
                                        AOT.add)
                ou = sc.tile([P, nf], BF16, tag="ou")
                nc.vector.tensor_tensor(ou[:, :], over[:, :], under[:, :],
                                        AOT.add)
                Acoef = sc.tile([P, nf], BF16, tag="Acoef")
                nc.any.tensor_scalar(Acoef[:, :], ou[:, :], -1.0, 1.0,
                                     AOT.mult, AOT.add)

                # --- value-level muxes ---
                VV = sc.tile([P, 2 * nf], F32, tag="VV")
                Vexp = VV[:, 0:nf]
                Vm = VV[:, nf:2 * nf]
                vx = sc.tile([P, nf], F32, tag="vx")
                nc.vector.tensor_tensor(vx[:, :], T2[:, :], Acoef[:, :],
                                        AOT.mult)
                nc.vector.scalar_tensor_tensor(Vexp, over[:, :], 255.0,
                                               vx[:, :], AOT.mult, AOT.add)
                nv = sc.tile([P, nf], BF16, tag="nv")
                nc.any.tensor_scalar(nv[:, :], nan2[:, :], 2.0, float(2 ** 22),
                                     AOT.is_ge, AOT.mult)
                mx = sc.tile([P, nf], F32, tag="mx")
                nc.vector.tensor_tensor(mx[:, :], Mval[:, :], Acoef[:, :],
                                        AOT.mult)
                nc.vector.tensor_tensor(Vm, mx[:, :], nv[:, :], AOT.add)

                # --- bit extraction (int32); one wide convert for both ---
                VVi = sc.tile([P, 2 * nf], I32, tag="VVi")
                nc.any.tensor_copy(VVi[:, :], VV[:, :])
                Vexpi = VVi[:, 0:nf]
                Vmi = VVi[:, nf:2 * nf]

                yti = mid.tile([P, nf * D_OUT], I32, tag="yti")
                ytv = yti[:, :].rearrange("p (n d) -> p n d", d=D_OUT)
                # sign (col 0): f32 -> i32 convert copy
                nc.any.tensor_copy(ytv[:, :, 0], sign)
                # exp cols 1..8 = T2 bits 7..0
                for c in range(1, 9):
                    j = 8 - c
                    nc.any.tensor_scalar(ytv[:, :, c], Vexpi, j, 1,
                                         AOT.logical_shift_right,
                                         AOT.bitwise_and)
                # mant cols 9..31 = M bits 22..0
                for c in range(9, 32):
                    j = 31 - c
                    nc.any.tensor_scalar(ytv[:, :, c], Vmi, j, 1,
                                         AOT.logical_shift_right,
                                         AOT.bitwise_and)

                # int32 -> f32 convert in place via bitcast view, then DMA out
                ytf = yti[:, :].bitcast(F32)
                nc.any.tensor_copy(ytf, yti[:, :])
                nc.sync.dma_start(
                    y_r[:, off * D_OUT:(off + nf) * D_OUT], ytf)
                off += nf

    nc.compile()
    return nc


def _get_nc():
    if "nc" not in _CACHE:
        _CACHE["nc"] = _build()
    return _CACHE["nc"]


def kernel(fp64_pulse: np.ndarray) -> np.ndarray:
    x = np.ascontiguousarray(fp64_pulse, dtype=np.float32)
    assert x.shape == (B, D_IN)
    nc = _get_nc()
    in_maps = [
        {"x": x[c * B_CORE:(c + 1) * B_CORE], "w": WCONST}
        for c in range(N_CORES)
    ]
    res = run_bass_kernel_spmd(nc, in_maps, core_ids=list(range(N_CORES)))
    return np.concatenate([r["y"] for r in res.results], axis=0)
